# revision 21
# baseline (speedup 1.0000x reference)
"""HSMNet cost-volume + disparity softmax-regression on 8 Trainium2 NeuronCores.

Reference computation (per batch b):
  cost[c,d,h,w] = |ref[c,h,w] - tgt[c,h,w-d]| for w>=d else 0
  cost_agg[d,h,w] = sum_c cost
  pred[h,w] = sum_d d * softmax_d(cost_agg)

Sharding: 8 cores = 4 batches x 2 h-halves (40 rows of 80 each). Each core
processes its [32, 40, 160] slice fully fused on-chip:
  - pixels flattened to 6400, split into 4 quarters of 1600; disparity
    handled as 6 blocks of 4 d's packed with the 32 channels into 128 SBUF
    partitions (partition = c + 32*j', disparity d = 4*blk + (3-j')).
  - inputs are cast to f16 on the host; each quarter is loaded by ONE
    HWDGE DMA per tensor that also replicates to the 4 partition groups:
    ref via a 0-stride group dim, tgt via a +1-stride group dim that bakes
    the per-group shift (group j' holds tgt shifted by 3-j'), so a single
    DVE tensor_tensor subtract with a uniform column offset produces diffs
    for 4 disparities at once. Loads are emitted inside the quarter loop so
    DMA-completion semaphore thresholds stay per-quarter (emitting them all
    up front made the first subtract wait for every load).
  - abs per (quarter, block): DVE uint16 bitand / ACT Abs, split to balance
    the two engines (gpsimd compute ops don't pass codegen here)
  - channel reduction via TensorE matmul with a 0/1 lhsT -> PSUM [24, *];
    the w<d validity mask is applied as an extra accumulating matmul that
    adds -100 to invalid (d, w mod 160) entries (exp -> ~0)
  - softmax: ACT Exp evacuates PSUM -> E[96,1600] bf16 (quarters stacked on
    partitions), TensorE contracts with [ones; d] weights -> den/num [8,1600]
  - host divides num/den (invalid entries' exp(0)=1 reference terms are
    dropped; they are ~1e-15 of den for randn-scale inputs)
"""
import os
import sys
import threading

for _p in ("/opt/trn_rl_repo",):
    if os.path.isdir(_p) and _p not in sys.path:
        sys.path.insert(0, _p)

import numpy as np
import ml_dtypes

import concourse.bacc as bacc
import concourse.mybir as mybir
from concourse.tile import TileContext
from concourse.bass_utils import run_bass_kernel_spmd

dt = mybir.dt

# problem shape (hardcoded per spec)
B, C, H, W = 4, 32, 80, 160
D = 24
HP = H // 2            # rows per core
PIX = HP * W           # 6400 pixels per core
NB = D // 4            # 6 disparity blocks of 4
NQ = 4                 # pixel quarters
QW = PIX // NQ         # 1600
CH = 400               # matmul chunk (512-aligned in PSUM)
PAD = 24               # front columns in tgt quarter tiles (look-back window)
TQW = PAD + QW         # tgt quarter tile width (1624)
LP = PAD + 3           # 27 leading zero cols baked into the DRAM tgt tensor
N_CORES = 8
MBIG = -100.0          # mask penalty added to invalid cost entries

# abs engine per (quarter, block) index 0..23: "dve" = uint16 bitand,
# "act" = scalar engine Abs. 13 dve / 11 act balances DVE ~28us vs ACT
# ~27us (ACT also runs the exps + final copy and starts later); the last
# unit is dve so the tail chain isn't gated on a slow ACT Abs.
_DEF_ABS = ["act", "dve"] * 9 + ["dve"] * 6
ABS_ENGINES = os.environ.get("HSM_ABS", ",".join(_DEF_ABS)).split(",")
SUBCH = int(os.environ.get("HSM_SUBCH", "1600"))     # DVE subtract chunk
ABSCH = int(os.environ.get("HSM_ABSCH", "1600"))     # DVE bitand chunk
DIFF_BUFS = int(os.environ.get("HSM_DIFF_BUFS", "16"))


def _build_program():
    nc = bacc.Bacc("TRN2", target_bir_lowering=False)
    # host pre-replicates both tensors to 128 rows (partition c + 32j'):
    # ref_rep[c+32j', x] = ref[c, x]; tgt_rep[c+32j', x] = tgt_pad[c, x+j']
    # where tgt_pad has LP leading zeros. Plain [128, N] DMAs then spread
    # across all 16 SDMA engines (broadcast APs serialized onto 4).
    ref_h = nc.dram_tensor("ref", [128, PIX], dt.float16, kind="ExternalInput")
    tgt_h = nc.dram_tensor("tgt", [128, LP + PIX], dt.float16, kind="ExternalInput")
    lred_h = nc.dram_tensor("lred", [128, NB * D], dt.float16, kind="ExternalInput")
    lnd_h = nc.dram_tensor("lnd", [128, 8], dt.bfloat16, kind="ExternalInput")
    mbin_h = nc.dram_tensor("mbin", [D, QW], dt.float16, kind="ExternalInput")
    i24n_h = nc.dram_tensor("i24n", [D, D], dt.float16, kind="ExternalInput")
    out_h = nc.dram_tensor("out", [8, NQ * CH], dt.float32, kind="ExternalOutput")

    with TileContext(nc) as tc:
        with tc.tile_pool(name="const", bufs=1) as cpool, \
             tc.tile_pool(name="rep", bufs=1) as rpool, \
             tc.tile_pool(name="diffp", bufs=DIFF_BUFS) as dpool, \
             tc.tile_pool(name="ep", bufs=1) as epool:
            lred_sb = cpool.tile([128, NB * D], dt.float16)
            lnd_sb = cpool.tile([128, 8], dt.bfloat16)
            mbin_sb = cpool.tile([D, QW], dt.float16)
            i24n_sb = cpool.tile([D, D], dt.float16)

            refq = [rpool.tile([128, QW], dt.float16, name=f"refq{q}")
                    for q in range(NQ)]
            tgtq = [rpool.tile([128, TQW], dt.float16, name=f"tgtq{q}")
                    for q in range(NQ)]
            # E split into column halves (separate tiles) so the tail
            # nd/copy/out pipeline for half a isn't dep-gated on exp3b
            Eh = [epool.tile([128, QW // 2], dt.bfloat16, name=f"E{i}")
                  for i in range(2)]

            # E rows 24-31 of each 32-row group are never written by exp;
            # zero them so the num/den matmul reads no garbage
            nc.gpsimd.memset(Eh[0][:], 0.0)
            nc.gpsimd.memset(Eh[1][:], 0.0)

            with tc.tile_pool(name="cost", bufs=2, space="PSUM") as qpool:
                for q in range(NQ):
                    c0 = QW * q
                    # ---- loads for this quarter ----
                    # q0's pair goes on the sync HWDGE ring (the only ring
                    # fanning out to all 16 SDMA engines; the scalar ring
                    # gets 4, gpsimd 8). Consts ride the scalar ring before
                    # ACT compute begins; later refs ride gpsimd.
                    nc.sync.dma_start(tgtq[q][:], tgt_h[:, c0:c0 + TQW])
                    if q == 0:
                        nc.sync.dma_start(refq[q][:], ref_h[:, c0:c0 + QW])
                        nc.scalar.dma_start(lred_sb[:], lred_h[:])
                        nc.scalar.dma_start(mbin_sb[:], mbin_h[:])
                        nc.scalar.dma_start(i24n_sb[:], i24n_h[:])
                        nc.scalar.dma_start(lnd_sb[:], lnd_h[:])
                    else:
                        nc.gpsimd.dma_start(refq[q][:], ref_h[:, c0:c0 + QW])

                    # ---- compute for this quarter ----
                    diffs_q3 = []
                    cost = qpool.tile([D, 2048], dt.float32, tag="cost",
                                      name=f"cost_{q}")
                    # mask matmul opens each chunk's accumulation group:
                    # adds -100 at invalid (d, w mod 160 < d) positions
                    for cc in range(4):
                        nc.tensor.matmul(
                            cost[:, 512 * cc:512 * cc + CH],
                            i24n_sb[:], mbin_sb[:, CH * cc:CH * cc + CH],
                            start=True, stop=False)

                    for b in range(NB):
                        diff = dpool.tile([128, QW], dt.float16, tag="diff",
                                          name=f"diff_{q}_{b}")
                        # diff[c+32j', p] = ref[c, p] - tgt[c, p - 4b - (3-j')]
                        t0 = PAD - 4 * b
                        for x0 in range(0, QW, SUBCH):
                            x1 = min(x0 + SUBCH, QW)
                            nc.vector.tensor_tensor(
                                diff[:, x0:x1], refq[q][:, x0:x1],
                                tgtq[q][:, t0 + x0:t0 + x1],
                                mybir.AluOpType.subtract)
                        eng = ABS_ENGINES[(q * NB + b) % len(ABS_ENGINES)]
                        if eng == "dve":
                            for x0 in range(0, QW, ABSCH):
                                x1 = min(x0 + ABSCH, QW)
                                du = diff[:, x0:x1].bitcast(dt.uint16)
                                nc.vector.tensor_scalar(
                                    du, du, 0x7FFF, None,
                                    mybir.AluOpType.bitwise_and)
                        else:
                            nc.scalar.activation(diff[:], diff[:],
                                                 mybir.ActivationFunctionType.Abs)
                        if q < NQ - 1:
                            for cc in range(4):
                                nc.tensor.matmul(
                                    cost[:, 512 * cc:512 * cc + CH],
                                    lred_sb[:, D * b:D * (b + 1)],
                                    diff[:, CH * cc:CH * cc + CH],
                                    start=False, stop=(b == NB - 1))
                        else:
                            diffs_q3.append(diff)
                    if q == NQ - 1:
                        # last quarter: chunk-major so the first column
                        # half's accumulation closes as soon as the last
                        # diff lands, letting exp3a start ~1.5us earlier
                        for cc in range(4):
                            for b in range(NB):
                                nc.tensor.matmul(
                                    cost[:, 512 * cc:512 * cc + CH],
                                    lred_sb[:, D * b:D * (b + 1)],
                                    diffs_q3[b][:, CH * cc:CH * cc + CH],
                                    start=False, stop=(b == NB - 1))
                    # exp evacuate PSUM -> E bf16 (strided 512 -> packed 400),
                    # one op per column half
                    srcv = cost[:].rearrange("p (k x) -> p k x", k=4)[:, :, 0:CH]
                    for i in range(2):
                        dst = Eh[i][32 * q:32 * q + D, :] \
                            .rearrange("p (k x) -> p k x", x=CH)
                        nc.scalar.activation(dst, srcv[:, 2 * i:2 * i + 2, :],
                                             mybir.ActivationFunctionType.Exp)

            with tc.tile_pool(name="nd", bufs=2, space="PSUM") as npool:
                for i in range(2):
                    nd = npool.tile([8, 1024], dt.float32, tag="nd",
                                    name=f"nd{i}")
                    for cc in range(2):
                        nc.tensor.matmul(
                            nd[:, 512 * cc:512 * cc + CH], lnd_sb[:],
                            Eh[i][:, CH * cc:CH * (cc + 1)],
                            start=True, stop=True)
                    ndsrc = nd[:].rearrange("p (k x) -> p k x", k=2)[:, :, 0:CH]
                    out_sb = epool.tile([8, 2 * CH], dt.float32,
                                        name=f"osb{i}")
                    nc.scalar.activation(
                        out_sb[:].rearrange("p (k x) -> p k x", x=CH), ndsrc,
                        mybir.ActivationFunctionType.Copy)
                    nc.sync.dma_start(out_h[:, 2 * CH * i:2 * CH * (i + 1)],
                                      out_sb[:])

    nc.compile()
    return nc


def _host_constants():
    # lred: partition c + 32j' -> disparity d = 4b + (3-j') for block b
    lred = np.zeros((128, NB * D), np.float16)
    for b in range(NB):
        for jp in range(4):
            for c in range(C):
                lred[c + 32 * jp, D * b + 4 * b + (3 - jp)] = 1.0

    # lnd: den col q sums quarter q's rows; num col 4+q weights by d
    lnd = np.zeros((128, 8), np.float32)
    for q in range(4):
        for d in range(D):
            lnd[d + 32 * q, q] = 1.0
            lnd[d + 32 * q, 4 + q] = d
    lnd = lnd.astype(ml_dtypes.bfloat16)

    # mbin[dd, x] = 1 where (x mod 160) < dd (invalid); i24n = diag(MBIG)
    w = np.arange(W, dtype=np.int64)
    dvals = np.arange(D, dtype=np.int64)
    mbin = (np.tile(w, QW // W)[None, :] < dvals[:, None]).astype(np.float16)
    i24n = (np.eye(D) * MBIG).astype(np.float16)
    return lred, lnd, mbin, i24n


_lock = threading.Lock()
_cache = {}


def _get_program():
    with _lock:
        if "nc" not in _cache:
            _cache["nc"] = _build_program()
            _cache["consts"] = _host_constants()
        return _cache["nc"], _cache["consts"]


def _run(refimg_fea, targetimg_fea, trace=False):
    nc, (lred, lnd, mbin, i24n) = _get_program()
    ref = np.asarray(refimg_fea, dtype=np.float32).astype(np.float16)
    tgt = np.asarray(targetimg_fea, dtype=np.float32).astype(np.float16)
    in_maps = []
    for core in range(N_CORES):
        b, hh = core // 2, core % 2
        refc = ref[b, :, HP * hh:HP * (hh + 1), :].reshape(C, PIX)
        tgtc = tgt[b, :, HP * hh:HP * (hh + 1), :].reshape(C, PIX)
        # tgt_pad: LP leading zeros (w<d left edge), 3 trailing for shifts
        tgt_pad = np.zeros((C, LP + PIX + 3), np.float16)
        tgt_pad[:, LP:LP + PIX] = tgtc
        ref_rep = np.tile(refc, (4, 1))                      # [128, PIX]
        tgt_rep = np.empty((128, LP + PIX), np.float16)      # [128, LP+PIX]
        for jp in range(4):
            tgt_rep[32 * jp:32 * jp + 32, :] = tgt_pad[:, jp:jp + LP + PIX]
        in_maps.append({
            "ref": ref_rep, "tgt": tgt_rep,
            "lred": lred, "lnd": lnd, "mbin": mbin, "i24n": i24n,
        })
    res = run_bass_kernel_spmd(nc, in_maps, core_ids=list(range(N_CORES)),
                               trace=trace)
    out = np.empty((B, H, W), np.float32)
    for core in range(N_CORES):
        b, hh = core // 2, core % 2
        nd = res.results[core]["out"]          # [8, 1600]: den q rows 0-3, num rows 4-7
        pred = nd[4:8] / nd[0:4]               # [4, 1600]
        out[b, HP * hh:HP * (hh + 1), :] = pred.reshape(HP, W)
    return out, res


def kernel(refimg_fea, targetimg_fea, maxdisp):
    assert int(maxdisp) == D, f"kernel hardcodes maxdisp={D}, got {maxdisp}"
    out, _ = _run(refimg_fea, targetimg_fea)
    return out


# revision 23
# speedup vs baseline: 1.0768x; 1.0768x over previous
"""HSMNet cost-volume + disparity softmax-regression on 8 Trainium2 NeuronCores.

Reference computation (per batch b):
  cost[c,d,h,w] = |ref[c,h,w] - tgt[c,h,w-d]| for w>=d else 0
  cost_agg[d,h,w] = sum_c cost
  pred[h,w] = sum_d d * softmax_d(cost_agg)

Sharding: 8 cores = 4 batches x 2 h-halves (40 rows of 80 each). Each core
processes its [32, 40, 160] slice fully fused on-chip:
  - pixels flattened to 6400, split into 4 quarters of 1600; disparity
    handled as 6 blocks of 4 d's packed with the 32 channels into 128 SBUF
    partitions (partition = c + 32*j', disparity d = 4*blk + (3-j')).
  - inputs are cast to f16 on the host; each quarter is loaded by ONE
    HWDGE DMA per tensor that also replicates to the 4 partition groups:
    ref via a 0-stride group dim, tgt via a +1-stride group dim that bakes
    the per-group shift (group j' holds tgt shifted by 3-j'), so a single
    DVE tensor_tensor subtract with a uniform column offset produces diffs
    for 4 disparities at once. Loads are emitted inside the quarter loop so
    DMA-completion semaphore thresholds stay per-quarter (emitting them all
    up front made the first subtract wait for every load).
  - abs per (quarter, block): DVE uint16 bitand / ACT Abs, split to balance
    the two engines (gpsimd compute ops don't pass codegen here)
  - channel reduction via TensorE matmul with a 0/1 lhsT -> PSUM [24, *];
    the w<d validity mask is applied as an extra accumulating matmul that
    adds -100 to invalid (d, w mod 160) entries (exp -> ~0)
  - softmax: ACT Exp evacuates PSUM -> E[96,1600] bf16 (quarters stacked on
    partitions), TensorE contracts with [ones; d] weights -> den/num [8,1600]
  - host divides num/den (invalid entries' exp(0)=1 reference terms are
    dropped; they are ~1e-15 of den for randn-scale inputs)
"""
import os
import sys
import threading

for _p in ("/opt/trn_rl_repo",):
    if os.path.isdir(_p) and _p not in sys.path:
        sys.path.insert(0, _p)

import numpy as np
import ml_dtypes

import concourse.bacc as bacc
import concourse.mybir as mybir
from concourse.tile import TileContext
from concourse.bass_utils import run_bass_kernel_spmd

dt = mybir.dt

# problem shape (hardcoded per spec)
B, C, H, W = 4, 32, 80, 160
D = 24
HP = H // 2            # rows per core
PIX = HP * W           # 6400 pixels per core
NB = D // 4            # 6 disparity blocks of 4
NQ = 4                 # pixel quarters
QW = PIX // NQ         # 1600
CH = 400               # matmul chunk (512-aligned in PSUM)
PAD = 24               # front columns in tgt quarter tiles (look-back window)
TQW = PAD + QW         # tgt quarter tile width (1624)
LP = PAD + 3           # 27 leading zero cols baked into the DRAM tgt tensor
N_CORES = 8
MBIG = -100.0          # mask penalty added to invalid cost entries

# abs engine per (quarter, block) index 0..23: "dve" = uint16 bitand,
# "act" = scalar engine Abs. 13 dve / 11 act balances DVE ~28us vs ACT
# ~27us (ACT also runs the exps + final copy and starts later); the last
# unit is dve so the tail chain isn't gated on a slow ACT Abs.
_DEF_ABS = ["act", "dve"] * 9 + ["dve"] * 6
ABS_ENGINES = os.environ.get("HSM_ABS", ",".join(_DEF_ABS)).split(",")
SUBCH = int(os.environ.get("HSM_SUBCH", "1600"))     # DVE subtract chunk
ABSCH = int(os.environ.get("HSM_ABSCH", "1600"))     # DVE bitand chunk
DIFF_BUFS = int(os.environ.get("HSM_DIFF_BUFS", "16"))


def _build_program():
    nc = bacc.Bacc("TRN2", target_bir_lowering=False)
    # host pre-replicates both tensors to 128 rows (partition c + 32j'):
    # ref_rep[c+32j', x] = ref[c, x]; tgt_rep[c+32j', x] = tgt_pad[c, x+j']
    # where tgt_pad has LP leading zeros. Plain [128, N] DMAs then spread
    # across all 16 SDMA engines (broadcast APs serialized onto 4).
    ref_h = nc.dram_tensor("ref", [128, PIX], dt.float16, kind="ExternalInput")
    tgt_h = nc.dram_tensor("tgt", [128, LP + PIX], dt.float16, kind="ExternalInput")
    lred_h = nc.dram_tensor("lred", [128, NB * D], dt.float16, kind="ExternalInput")
    lnd_h = nc.dram_tensor("lnd", [128, 8], dt.bfloat16, kind="ExternalInput")
    mbin_h = nc.dram_tensor("mbin", [D, QW], dt.float16, kind="ExternalInput")
    i24n_h = nc.dram_tensor("i24n", [D, D], dt.float16, kind="ExternalInput")
    out_h = nc.dram_tensor("out", [8, NQ * CH], dt.float32, kind="ExternalOutput")

    with TileContext(nc) as tc:
        with tc.tile_pool(name="const", bufs=1) as cpool, \
             tc.tile_pool(name="rep", bufs=1) as rpool, \
             tc.tile_pool(name="diffp", bufs=DIFF_BUFS) as dpool, \
             tc.tile_pool(name="ep", bufs=1) as epool:
            lred_sb = cpool.tile([128, NB * D], dt.float16)
            lnd_sb = cpool.tile([128, 8], dt.bfloat16)
            mbin_sb = cpool.tile([D, QW], dt.float16)
            i24n_sb = cpool.tile([D, D], dt.float16)

            refq = [rpool.tile([128, QW], dt.float16, name=f"refq{q}")
                    for q in range(NQ)]
            tgtq = [rpool.tile([128, TQW], dt.float16, name=f"tgtq{q}")
                    for q in range(NQ)]
            # E split into column halves (separate tiles) so the tail
            # nd/copy/out pipeline for half a isn't dep-gated on exp3b
            Eh = [epool.tile([128, QW // 2], dt.bfloat16, name=f"E{i}")
                  for i in range(2)]

            # E rows 24-31 of each 32-row group are never written by exp;
            # zero them so the num/den matmul reads no garbage
            nc.gpsimd.memset(Eh[0][:], 0.0)
            nc.gpsimd.memset(Eh[1][:], 0.0)

            with tc.tile_pool(name="cost", bufs=2, space="PSUM") as qpool:
                for q in range(NQ):
                    c0 = QW * q
                    # ---- loads for this quarter ----
                    # q0's pair goes on the sync HWDGE ring (the only ring
                    # fanning out to all 16 SDMA engines; the scalar ring
                    # gets 4, gpsimd 8). Consts ride the scalar ring before
                    # ACT compute begins; later refs ride gpsimd.
                    nc.sync.dma_start(tgtq[q][:], tgt_h[:, c0:c0 + TQW])
                    if q == 0:
                        nc.sync.dma_start(refq[q][:], ref_h[:, c0:c0 + QW])
                        nc.scalar.dma_start(lred_sb[:], lred_h[:])
                        nc.scalar.dma_start(mbin_sb[:], mbin_h[:])
                        nc.scalar.dma_start(i24n_sb[:], i24n_h[:])
                        nc.scalar.dma_start(lnd_sb[:], lnd_h[:])
                    else:
                        nc.gpsimd.dma_start(refq[q][:], ref_h[:, c0:c0 + QW])

                    # ---- compute for this quarter ----
                    diffs_q3 = []
                    cost = qpool.tile([D, 2048], dt.float32, tag="cost",
                                      name=f"cost_{q}")
                    # mask matmul opens each chunk's accumulation group:
                    # adds -100 at invalid (d, w mod 160 < d) positions
                    for cc in range(4):
                        nc.tensor.matmul(
                            cost[:, 512 * cc:512 * cc + CH],
                            i24n_sb[:], mbin_sb[:, CH * cc:CH * cc + CH],
                            start=True, stop=False)

                    for b in range(NB):
                        diff = dpool.tile([128, QW], dt.float16, tag="diff",
                                          name=f"diff_{q}_{b}")
                        # diff[c+32j', p] = ref[c, p] - tgt[c, p - 4b - (3-j')]
                        t0 = PAD - 4 * b
                        for x0 in range(0, QW, SUBCH):
                            x1 = min(x0 + SUBCH, QW)
                            nc.vector.tensor_tensor(
                                diff[:, x0:x1], refq[q][:, x0:x1],
                                tgtq[q][:, t0 + x0:t0 + x1],
                                mybir.AluOpType.subtract)
                        eng = ABS_ENGINES[(q * NB + b) % len(ABS_ENGINES)]
                        if eng == "dve":
                            for x0 in range(0, QW, ABSCH):
                                x1 = min(x0 + ABSCH, QW)
                                du = diff[:, x0:x1].bitcast(dt.uint16)
                                nc.vector.tensor_scalar(
                                    du, du, 0x7FFF, None,
                                    mybir.AluOpType.bitwise_and)
                        else:
                            nc.scalar.activation(diff[:], diff[:],
                                                 mybir.ActivationFunctionType.Abs)
                        for cc in range(4):
                            nc.tensor.matmul(
                                cost[:, 512 * cc:512 * cc + CH],
                                lred_sb[:, D * b:D * (b + 1)],
                                diff[:, CH * cc:CH * cc + CH],
                                start=False, stop=(b == NB - 1))
                    # exp evacuate PSUM -> E bf16 (strided 512 -> packed 400),
                    # one op per column half
                    srcv = cost[:].rearrange("p (k x) -> p k x", k=4)[:, :, 0:CH]
                    for i in range(2):
                        dst = Eh[i][32 * q:32 * q + D, :] \
                            .rearrange("p (k x) -> p k x", x=CH)
                        nc.scalar.activation(dst, srcv[:, 2 * i:2 * i + 2, :],
                                             mybir.ActivationFunctionType.Exp)

            with tc.tile_pool(name="nd", bufs=2, space="PSUM") as npool:
                for i in range(2):
                    nd = npool.tile([8, 1024], dt.float32, tag="nd",
                                    name=f"nd{i}")
                    for cc in range(2):
                        nc.tensor.matmul(
                            nd[:, 512 * cc:512 * cc + CH], lnd_sb[:],
                            Eh[i][:, CH * cc:CH * (cc + 1)],
                            start=True, stop=True)
                    ndsrc = nd[:].rearrange("p (k x) -> p k x", k=2)[:, :, 0:CH]
                    out_sb = epool.tile([8, 2 * CH], dt.float32,
                                        name=f"osb{i}")
                    nc.scalar.activation(
                        out_sb[:].rearrange("p (k x) -> p k x", x=CH), ndsrc,
                        mybir.ActivationFunctionType.Copy)
                    nc.sync.dma_start(out_h[:, 2 * CH * i:2 * CH * (i + 1)],
                                      out_sb[:])

    nc.compile()
    return nc


def _host_constants():
    # lred: partition c + 32j' -> disparity d = 4b + (3-j') for block b
    lred = np.zeros((128, NB * D), np.float16)
    for b in range(NB):
        for jp in range(4):
            for c in range(C):
                lred[c + 32 * jp, D * b + 4 * b + (3 - jp)] = 1.0

    # lnd: den col q sums quarter q's rows; num col 4+q weights by d
    lnd = np.zeros((128, 8), np.float32)
    for q in range(4):
        for d in range(D):
            lnd[d + 32 * q, q] = 1.0
            lnd[d + 32 * q, 4 + q] = d
    lnd = lnd.astype(ml_dtypes.bfloat16)

    # mbin[dd, x] = 1 where (x mod 160) < dd (invalid); i24n = diag(MBIG)
    w = np.arange(W, dtype=np.int64)
    dvals = np.arange(D, dtype=np.int64)
    mbin = (np.tile(w, QW // W)[None, :] < dvals[:, None]).astype(np.float16)
    i24n = (np.eye(D) * MBIG).astype(np.float16)
    return lred, lnd, mbin, i24n


_lock = threading.Lock()
_cache = {}


def _get_program():
    with _lock:
        if "nc" not in _cache:
            _cache["nc"] = _build_program()
            _cache["consts"] = _host_constants()
        return _cache["nc"], _cache["consts"]


def _run(refimg_fea, targetimg_fea, trace=False):
    nc, (lred, lnd, mbin, i24n) = _get_program()
    ref = np.asarray(refimg_fea, dtype=np.float32).astype(np.float16)
    tgt = np.asarray(targetimg_fea, dtype=np.float32).astype(np.float16)
    in_maps = []
    for core in range(N_CORES):
        b, hh = core // 2, core % 2
        refc = ref[b, :, HP * hh:HP * (hh + 1), :].reshape(C, PIX)
        tgtc = tgt[b, :, HP * hh:HP * (hh + 1), :].reshape(C, PIX)
        # tgt_pad: LP leading zeros (w<d left edge), 3 trailing for shifts
        tgt_pad = np.zeros((C, LP + PIX + 3), np.float16)
        tgt_pad[:, LP:LP + PIX] = tgtc
        ref_rep = np.tile(refc, (4, 1))                      # [128, PIX]
        tgt_rep = np.empty((128, LP + PIX), np.float16)      # [128, LP+PIX]
        for jp in range(4):
            tgt_rep[32 * jp:32 * jp + 32, :] = tgt_pad[:, jp:jp + LP + PIX]
        in_maps.append({
            "ref": ref_rep, "tgt": tgt_rep,
            "lred": lred, "lnd": lnd, "mbin": mbin, "i24n": i24n,
        })
    res = run_bass_kernel_spmd(nc, in_maps, core_ids=list(range(N_CORES)),
                               trace=trace)
    out = np.empty((B, H, W), np.float32)
    for core in range(N_CORES):
        b, hh = core // 2, core % 2
        nd = res.results[core]["out"]          # [8, 1600]: den q rows 0-3, num rows 4-7
        pred = nd[4:8] / nd[0:4]               # [4, 1600]
        out[b, HP * hh:HP * (hh + 1), :] = pred.reshape(HP, W)
    return out, res


def kernel(refimg_fea, targetimg_fea, maxdisp):
    assert int(maxdisp) == D, f"kernel hardcodes maxdisp={D}, got {maxdisp}"
    out, _ = _run(refimg_fea, targetimg_fea)
    return out


# revision 24
# speedup vs baseline: 1.0959x; 1.0177x over previous
"""HSMNet cost-volume + disparity softmax-regression on 8 Trainium2 NeuronCores.

Reference computation (per batch b):
  cost[c,d,h,w] = |ref[c,h,w] - tgt[c,h,w-d]| for w>=d else 0
  cost_agg[d,h,w] = sum_c cost
  pred[h,w] = sum_d d * softmax_d(cost_agg)

Sharding: 8 cores = 4 batches x 2 h-halves (40 rows of 80 each). Each core
processes its [32, 40, 160] slice fully fused on-chip:
  - pixels flattened to 6400, split into 4 quarters of 1600; disparity
    handled as 6 blocks of 4 d's packed with the 32 channels into 128 SBUF
    partitions (partition = c + 32*j', disparity d = 4*blk + (3-j')).
  - inputs are cast to f16 on the host; each quarter is loaded by ONE
    HWDGE DMA per tensor that also replicates to the 4 partition groups:
    ref via a 0-stride group dim, tgt via a +1-stride group dim that bakes
    the per-group shift (group j' holds tgt shifted by 3-j'), so a single
    DVE tensor_tensor subtract with a uniform column offset produces diffs
    for 4 disparities at once. Loads are emitted inside the quarter loop so
    DMA-completion semaphore thresholds stay per-quarter (emitting them all
    up front made the first subtract wait for every load).
  - abs per (quarter, block): DVE uint16 bitand / ACT Abs, split to balance
    the two engines (gpsimd compute ops don't pass codegen here)
  - channel reduction via TensorE matmul with a 0/1 lhsT -> PSUM [24, *];
    the w<d validity mask is applied as an extra accumulating matmul that
    adds -100 to invalid (d, w mod 160) entries (exp -> ~0)
  - softmax: ACT Exp evacuates PSUM -> E[96,1600] bf16 (quarters stacked on
    partitions), TensorE contracts with [ones; d] weights -> den/num [8,1600]
  - host divides num/den (invalid entries' exp(0)=1 reference terms are
    dropped; they are ~1e-15 of den for randn-scale inputs)
"""
import os
import sys
import threading

for _p in ("/opt/trn_rl_repo",):
    if os.path.isdir(_p) and _p not in sys.path:
        sys.path.insert(0, _p)

import numpy as np
import ml_dtypes

import concourse.bacc as bacc
import concourse.mybir as mybir
from concourse.tile import TileContext
from concourse.bass_utils import run_bass_kernel_spmd

dt = mybir.dt

# problem shape (hardcoded per spec)
B, C, H, W = 4, 32, 80, 160
D = 24
HP = H // 2            # rows per core
PIX = HP * W           # 6400 pixels per core
NB = D // 4            # 6 disparity blocks of 4
NQ = 4                 # pixel quarters
QW = PIX // NQ         # 1600
CH = 400               # matmul chunk (512-aligned in PSUM)
PAD = 24               # front columns in tgt quarter tiles (look-back window)
TQW = PAD + QW         # tgt quarter tile width (1624)
LP = PAD + 3           # 27 leading zero cols baked into the DRAM tgt tensor
N_CORES = 8
MBIG = -100.0          # mask penalty added to invalid cost entries

# abs engine per (quarter, block) index 0..23: "dve" = uint16 bitand,
# "act" = scalar engine Abs. 13 dve / 11 act balances DVE ~28us vs ACT
# ~27us (ACT also runs the exps + final copy and starts later); the last
# unit is dve so the tail chain isn't gated on a slow ACT Abs.
_DEF_ABS = ["act", "dve"] * 9 + ["dve"] * 6
ABS_ENGINES = os.environ.get("HSM_ABS", ",".join(_DEF_ABS)).split(",")
SUBCH = int(os.environ.get("HSM_SUBCH", "1600"))     # DVE subtract chunk
ABSCH = int(os.environ.get("HSM_ABSCH", "1600"))     # DVE bitand chunk
DIFF_BUFS = int(os.environ.get("HSM_DIFF_BUFS", "16"))


def _build_program():
    nc = bacc.Bacc("TRN2", target_bir_lowering=False)
    # host pre-replicates both tensors to 128 rows (partition c + 32j'):
    # ref_rep[c+32j', x] = ref[c, x]; tgt_rep[c+32j', x] = tgt_pad[c, x+j']
    # where tgt_pad has LP leading zeros. Plain [128, N] DMAs then spread
    # across all 16 SDMA engines (broadcast APs serialized onto 4).
    ref_h = nc.dram_tensor("ref", [128, PIX], dt.float16, kind="ExternalInput")
    tgt_h = nc.dram_tensor("tgt", [128, LP + PIX], dt.float16, kind="ExternalInput")
    lred_h = nc.dram_tensor("lred", [128, NB * D], dt.float16, kind="ExternalInput")
    lnd_h = nc.dram_tensor("lnd", [128, 8], dt.bfloat16, kind="ExternalInput")
    mbin_h = nc.dram_tensor("mbin", [D, QW], dt.float16, kind="ExternalInput")
    i24n_h = nc.dram_tensor("i24n", [D, D], dt.float16, kind="ExternalInput")
    out_h = nc.dram_tensor("out", [8, NQ * CH], dt.float32, kind="ExternalOutput")

    with TileContext(nc) as tc:
        with tc.tile_pool(name="const", bufs=1) as cpool, \
             tc.tile_pool(name="rep", bufs=1) as rpool, \
             tc.tile_pool(name="diffp", bufs=DIFF_BUFS) as dpool, \
             tc.tile_pool(name="ep", bufs=1) as epool:
            lred_sb = cpool.tile([128, NB * D], dt.float16)
            lnd_sb = cpool.tile([128, 8], dt.bfloat16)
            mbin_sb = cpool.tile([D, QW], dt.float16)
            i24n_sb = cpool.tile([D, D], dt.float16)

            refq = [rpool.tile([128, QW], dt.float16, name=f"refq{q}")
                    for q in range(NQ)]
            tgtq = [rpool.tile([128, TQW], dt.float16, name=f"tgtq{q}")
                    for q in range(NQ)]
            # E split into column halves (separate tiles) so the tail
            # nd/copy/out pipeline for half a isn't dep-gated on exp3b
            Eh = [epool.tile([128, QW // 2], dt.bfloat16, name=f"E{i}")
                  for i in range(2)]

            # E rows 24-31 of each 32-row group are never written by exp;
            # zero them so the num/den matmul reads no garbage
            nc.gpsimd.memset(Eh[0][:], 0.0)
            nc.gpsimd.memset(Eh[1][:], 0.0)

            with tc.tile_pool(name="cost", bufs=2, space="PSUM") as qpool:
                for q in range(NQ):
                    c0 = QW * q
                    # ---- loads for this quarter ----
                    # q0's pair goes on the sync HWDGE ring (the only ring
                    # fanning out to all 16 SDMA engines; the scalar ring
                    # gets 4, gpsimd 8). Consts ride the scalar ring before
                    # ACT compute begins; later refs ride gpsimd.
                    nc.sync.dma_start(tgtq[q][:], tgt_h[:, c0:c0 + TQW])
                    if q == 0:
                        nc.sync.dma_start(refq[q][:], ref_h[:, c0:c0 + QW])
                        nc.scalar.dma_start(lred_sb[:], lred_h[:])
                        nc.scalar.dma_start(mbin_sb[:], mbin_h[:])
                        nc.scalar.dma_start(i24n_sb[:], i24n_h[:])
                        nc.scalar.dma_start(lnd_sb[:], lnd_h[:])
                    else:
                        nc.gpsimd.dma_start(refq[q][:], ref_h[:, c0:c0 + QW])

                    # ---- compute for this quarter ----
                    cost = qpool.tile([D, 2048], dt.float32, tag="cost",
                                      name=f"cost_{q}")
                    # mask matmul opens each chunk's accumulation group:
                    # adds -100 at invalid (d, w mod 160 < d) positions
                    for cc in range(4):
                        nc.tensor.matmul(
                            cost[:, 512 * cc:512 * cc + CH],
                            i24n_sb[:], mbin_sb[:, CH * cc:CH * cc + CH],
                            start=True, stop=False)

                    for b in range(NB):
                        diff = dpool.tile([128, QW], dt.float16, tag="diff",
                                          name=f"diff_{q}_{b}")
                        # diff[c+32j', p] = ref[c, p] - tgt[c, p - 4b - (3-j')]
                        t0 = PAD - 4 * b
                        for x0 in range(0, QW, SUBCH):
                            x1 = min(x0 + SUBCH, QW)
                            nc.vector.tensor_tensor(
                                diff[:, x0:x1], refq[q][:, x0:x1],
                                tgtq[q][:, t0 + x0:t0 + x1],
                                mybir.AluOpType.subtract)
                        eng = ABS_ENGINES[(q * NB + b) % len(ABS_ENGINES)]
                        if eng == "dve":
                            for x0 in range(0, QW, ABSCH):
                                x1 = min(x0 + ABSCH, QW)
                                du = diff[:, x0:x1].bitcast(dt.uint16)
                                nc.vector.tensor_scalar(
                                    du, du, 0x7FFF, None,
                                    mybir.AluOpType.bitwise_and)
                        else:
                            nc.scalar.activation(diff[:], diff[:],
                                                 mybir.ActivationFunctionType.Abs)
                        for cc in range(4):
                            nc.tensor.matmul(
                                cost[:, 512 * cc:512 * cc + CH],
                                lred_sb[:, D * b:D * (b + 1)],
                                diff[:, CH * cc:CH * cc + CH],
                                start=False, stop=(b == NB - 1))
                    # exp evacuate PSUM -> E bf16 (strided 512 -> packed 400),
                    # one op per column half
                    srcv = cost[:].rearrange("p (k x) -> p k x", k=4)[:, :, 0:CH]
                    for i in range(2):
                        dst = Eh[i][32 * q:32 * q + D, :] \
                            .rearrange("p (k x) -> p k x", x=CH)
                        nc.scalar.activation(dst, srcv[:, 2 * i:2 * i + 2, :],
                                             mybir.ActivationFunctionType.Exp)

            with tc.tile_pool(name="nd", bufs=2, space="PSUM") as npool:
                for i in range(2):
                    nd = npool.tile([8, 1024], dt.float32, tag="nd",
                                    name=f"nd{i}")
                    for cc in range(2):
                        nc.tensor.matmul(
                            nd[:, 512 * cc:512 * cc + CH], lnd_sb[:],
                            Eh[i][:, CH * cc:CH * (cc + 1)],
                            start=True, stop=True)
                    ndsrc = nd[:].rearrange("p (k x) -> p k x", k=2)[:, :, 0:CH]
                    out_sb = epool.tile([8, 2 * CH], dt.float32,
                                        name=f"osb{i}")
                    nc.scalar.activation(
                        out_sb[:].rearrange("p (k x) -> p k x", x=CH), ndsrc,
                        mybir.ActivationFunctionType.Copy)
                    nc.sync.dma_start(out_h[:, 2 * CH * i:2 * CH * (i + 1)],
                                      out_sb[:])

    nc.compile()
    return nc


def _host_constants():
    # lred: partition c + 32j' -> disparity d = 4b + (3-j') for block b
    lred = np.zeros((128, NB * D), np.float16)
    for b in range(NB):
        for jp in range(4):
            for c in range(C):
                lred[c + 32 * jp, D * b + 4 * b + (3 - jp)] = 1.0

    # lnd: den col q sums quarter q's rows; num col 4+q weights by d
    lnd = np.zeros((128, 8), np.float32)
    for q in range(4):
        for d in range(D):
            lnd[d + 32 * q, q] = 1.0
            lnd[d + 32 * q, 4 + q] = d
    lnd = lnd.astype(ml_dtypes.bfloat16)

    # mbin[dd, x] = 1 where (x mod 160) < dd (invalid); i24n = diag(MBIG)
    w = np.arange(W, dtype=np.int64)
    dvals = np.arange(D, dtype=np.int64)
    mbin = (np.tile(w, QW // W)[None, :] < dvals[:, None]).astype(np.float16)
    i24n = (np.eye(D) * MBIG).astype(np.float16)
    return lred, lnd, mbin, i24n


_lock = threading.Lock()
_cache = {}


def _get_program():
    with _lock:
        if "nc" not in _cache:
            _cache["nc"] = _build_program()
            _cache["consts"] = _host_constants()
        return _cache["nc"], _cache["consts"]


def _run(refimg_fea, targetimg_fea, trace=False):
    nc, (lred, lnd, mbin, i24n) = _get_program()
    ref = np.asarray(refimg_fea, dtype=np.float32).astype(np.float16)
    tgt = np.asarray(targetimg_fea, dtype=np.float32).astype(np.float16)
    in_maps = []
    for core in range(N_CORES):
        b, hh = core // 2, core % 2
        refc = ref[b, :, HP * hh:HP * (hh + 1), :].reshape(C, PIX)
        tgtc = tgt[b, :, HP * hh:HP * (hh + 1), :].reshape(C, PIX)
        # tgt_pad: LP leading zeros (w<d left edge), 3 trailing for shifts
        tgt_pad = np.zeros((C, LP + PIX + 3), np.float16)
        tgt_pad[:, LP:LP + PIX] = tgtc
        ref_rep = np.tile(refc, (4, 1))                      # [128, PIX]
        tgt_rep = np.empty((128, LP + PIX), np.float16)      # [128, LP+PIX]
        for jp in range(4):
            tgt_rep[32 * jp:32 * jp + 32, :] = tgt_pad[:, jp:jp + LP + PIX]
        in_maps.append({
            "ref": ref_rep, "tgt": tgt_rep,
            "lred": lred, "lnd": lnd, "mbin": mbin, "i24n": i24n,
        })
    res = run_bass_kernel_spmd(nc, in_maps, core_ids=list(range(N_CORES)),
                               trace=trace)
    out = np.empty((B, H, W), np.float32)
    for core in range(N_CORES):
        b, hh = core // 2, core % 2
        nd = res.results[core]["out"]          # [8, 1600]: den q rows 0-3, num rows 4-7
        pred = nd[4:8] / nd[0:4]               # [4, 1600]
        out[b, HP * hh:HP * (hh + 1), :] = pred.reshape(HP, W)
    return out, res


def kernel(refimg_fea, targetimg_fea, maxdisp):
    assert int(maxdisp) == D, f"kernel hardcodes maxdisp={D}, got {maxdisp}"
    out, _ = _run(refimg_fea, targetimg_fea)
    return out
